# revision 56
# baseline (speedup 1.0000x reference)
"""MCCDecoderAttention Trainium2 kernel (8 NeuronCores).

Sharding: core = b*4 + g  (b in {0,1} batch, g in {0..3} head-group).
Each core computes attention for 3 heads of one batch, plus its partial
contribution to the output projection; the host sums the 4 partials per
batch and adds b_proj.

Device-side layout (per core):
  xT    [768, 2048]  x[b].T (feature-major, bf16)
  wqkT  [768, 384]   w_qkv.T columns [q_h0 q_h1 | k_h0 k_h1 | q_h2 k_h2] (bf16)
  wvT   [768, 192]   w_qkv.T v-columns [v_h0 v_h1 v_h2] (bf16)
  wpT   [192, 768]   w_proj.T rows for this core's 192 channels (bf16)
  yT    [768, 2048]  partial output (feature-major, bf16; host sums in f32)

Pipeline: S^T tiles come from matmul(lhsT=K_T, rhs=Q_T) with keys on
partitions; exp runs on ScalarE into bf16 A-tiles (~76us total -- the hard
engine floor).  A@V uses the A-tile as the *stationary* operand and
token-major V (+ones column for the softmax denominator L) as the 65-wide
*moving* operand, producing O token-major -- half the PE columns of the
feature-major orientation.  Per-query 1/L normalization and the decoder
mask's diagonal correction are per-partition-scalar vector ops; normalized
bf16 O tiles are PE-transposed back to feature-major for the output
projection.  The six (head, query-half) S^T streams are emitted
tile-interleaved with all other PE work (QKV groups, A@V, transposes,
projection) so ScalarE runs continuously from ~8us to the end; the last
head's A@V overlaps the final transposes + projection to shrink the tail.
"""

import functools
import os
import sys

for _p in ("/opt/trn_rl_repo", "/root/.axon_site/_ro/trn_rl_repo"):
    if os.path.isdir(_p) and _p not in sys.path:
        sys.path.insert(0, _p)

import numpy as np

import concourse.bacc as bacc
import concourse.tile as tile
from concourse import mybir
from concourse.masks import make_identity

N, C, D = 2048, 768, 64
NH = 3            # heads per core
CT = C // 128     # 6 contraction tiles
F32 = mybir.dt.float32
F32R = mybir.dt.float32r
BF16 = mybir.dt.bfloat16
EXP = mybir.ActivationFunctionType.Exp
MUL = mybir.AluOpType.mult
ADD = mybir.AluOpType.add

_last_results = None  # BassKernelResults of the most recent run (for test.py)


@functools.lru_cache(maxsize=4)
def _build(u: int):
    nc = bacc.Bacc(None, target_bir_lowering=False)
    xT = nc.dram_tensor("xT", [C, N], BF16, kind="ExternalInput")
    wqkT = nc.dram_tensor("wqkT", [C, 6 * D], BF16, kind="ExternalInput")
    wvT = nc.dram_tensor("wvT", [C, NH * D], BF16, kind="ExternalInput")
    wpT = nc.dram_tensor("wpT", [NH * D, C], BF16, kind="ExternalInput")
    yT = nc.dram_tensor("yT", [C, N], BF16, kind="ExternalOutput")

    kfull = N - u
    assert kfull % 128 == 0 and u % 128 == 0 and kfull >= 128
    T = kfull // 128          # key tiles in the dense region
    NQT = N // 128            # 16 query tiles
    uq0 = kfull // 128        # first query tile needing the diagonal term

    with nc.allow_low_precision(reason="f32r/bf16 staging"), \
         tile.TileContext(nc) as tc:
        with tc.tile_pool(name="persist", bufs=1) as P, \
             tc.tile_pool(name="o_ps", bufs=2, space="PSUM") as ops_, \
             tc.tile_pool(name="a_sb", bufs=36) as apool, \
             tc.tile_pool(name="out_sb", bufs=8) as out_sb, \
             tc.tile_pool(name="scr", bufs=4) as scr:
            xt = P.tile([128, CT, N], BF16)
            wqk = P.tile([128, CT, 6 * D], BF16)
            wv = P.tile([128, CT, NH * D], BF16)
            wp01 = P.tile([128, C], BF16)
            wp2 = P.tile([64, C], BF16)
            qT0 = P.tile([128, N], F32R)   # q heads 0,1 (rows 0-63 / 64-127)
            kT0 = P.tile([128, N], F32R)
            qT1 = P.tile([64, N], F32R)    # q head 2 (base partition 0)
            kT1 = P.tile([64, N], F32R)    # k head 2 (base partition 0)
            vsb = P.tile([128, NQT, NH, 65], BF16)  # token-major V + ones col
            opk01 = [P.tile([128, 128], BF16, name=f"opk01_{i}", tag=f"opk01_{i}")
                     for i in range(NQT)]
            opk2 = [P.tile([128, 64], BF16, name=f"opk2_{i}", tag=f"opk2_{i}")
                    for i in range(NQT)]
            ao01 = P.tile([128, N], BF16)
            ao2 = P.tile([64, N], BF16)
            ident = P.tile([128, 128], BF16)
            ones1 = P.tile([64, 1], F32)
            onesf = P.tile([128, NQT * NH], F32)

            # ---- input DMAs (few big transfers; order = availability order) ----
            xTr = xT.rearrange("(t p) n -> p t n", p=128)
            wqkTr = wqkT.rearrange("(t p) f -> p t f", p=128)
            nc.sync.dma_start(wqk[:, :, 0:256], wqkTr[:, :, 0:256])
            for ch in range(4):
                sl = slice(ch * 512, (ch + 1) * 512)
                nc.sync.dma_start(xt[:, :, sl], xTr[:, :, sl])
                if ch == 0:
                    nc.sync.dma_start(wqk[:, :, 256:384], wqkTr[:, :, 256:384])
                if ch == 1:
                    nc.sync.dma_start(wv[:], wvT.rearrange("(t p) f -> p t f", p=128))
            nc.sync.dma_start(wp01[:], wpT[0:128, :])
            nc.sync.dma_start(wp2[:], wpT[128:192, :])

            nc.vector.memset(onesf[:], 1.0)
            nc.vector.tensor_copy(
                vsb[:, :, :, 64:65],
                onesf[:].rearrange("p (a b x) -> p a b x", b=NH, x=1))
            nc.vector.tensor_copy(ones1[:], onesf[0:64, 0:1])
            make_identity(nc, ident)

            # ---- emission helpers ----
            heads = [  # (q tile, k tile, partition offset)
                (qT0, kT0, 0),
                (qT0, kT0, 64),
                (qT1, kT1, 0),
            ]
            fblocks = [slice(0, 128), slice(128, 256), slice(256, 384)]
            a_tiles = {}       # (h, half) -> list of a tiles
            qkv_pool = [None]  # boxed so helpers see the active pool

            def emit_qk(fi, ch, on_act=False):
                fc = fblocks[fi]
                sl = slice(ch * 512, (ch + 1) * 512)
                ps = qkv_pool[0].tile([128, 512], F32, tag="qk", name="qkps")
                for ct in range(CT):
                    nc.tensor.matmul(ps[:], wqk[:, ct, fc], xt[:, ct, sl],
                                     start=(ct == 0), stop=(ct == CT - 1))
                eng = nc.scalar.copy if on_act else nc.vector.tensor_copy
                if fi == 2:  # noqa: SIM108
                    # split: q2 from psum rows 0:64, k2 from rows 64:128
                    # (engine copies can shift partition base; verified on HW)
                    eng(qT1[:, sl], ps[0:64, :])
                    eng(kT1[:, sl], ps[64:128, :])
                else:
                    eng((qT0, kT0)[fi][:, sl], ps[:])

            def emit_v(nt):
                ps = qkv_pool[0].tile([128, 512], F32, tag="qk", name="vps")
                for ct in range(CT):
                    nc.tensor.matmul(ps[:, 0:NH * D],
                                     xt[:, ct, nt * 128:(nt + 1) * 128],
                                     wv[:, ct, :],
                                     start=(ct == 0), stop=(ct == CT - 1))
                nc.vector.tensor_copy(
                    vsb[:, nt, :, 0:64],
                    ps[:, 0:NH * D].rearrange("p (h x) -> p h x", x=64))

            st_pool = [None]

            def emit_st(h, half, t):
                qt_, kt_, bh = heads[h]
                st = st_pool[0].tile([128, 1024], F32, tag="st", name="st")
                for cc in range(2):
                    qsl = slice(half * 1024 + cc * 512, half * 1024 + cc * 512 + 512)
                    nc.tensor.matmul(st[:, cc * 512:(cc + 1) * 512],
                                     kt_[bh:bh + 64, t * 128:(t + 1) * 128],
                                     qt_[bh:bh + 64, qsl], start=True, stop=True)
                a = apool.tile([128, 1024], BF16, tag="a", name="a4")
                nc.scalar.activation(a[:], st[:], EXP, scale=0.125)
                a_tiles.setdefault((h, half), []).append(a)

            def emit_av(h, half, qt):
                """qt in 0..7 within this half; writes opk tiles (normalized bf16)."""
                qg = half * 8 + qt
                alist = a_tiles[(h, half)]
                qt_, kt_, bh = heads[h]
                o = ops_.tile([128, 66], F32, tag="o", name="o")
                for t in range(T):
                    nc.tensor.matmul(o[:, 0:65], alist[t][:, qt * 128:(qt + 1) * 128],
                                     vsb[:, t, h, :],
                                     start=(t == 0), stop=(t == T - 1))
                corr = qg >= uq0
                if corr:
                    qsl = slice(qg * 128, (qg + 1) * 128)
                    prod = scr.tile([64, 128], F32, tag="prod", name="prod")
                    # all-SBUF corr ops go to the otherwise-idle GpSimd
                    nc.gpsimd.tensor_mul(prod[:], qt_[bh:bh + 64, qsl],
                                         kt_[bh:bh + 64, qsl])
                    nc.tensor.matmul(o[:, 65:66], prod[:], ones1[:],
                                     start=True, stop=True, skip_group_check=True)
                    e = scr.tile([128, 1], F32, tag="e", name="e")
                    nc.scalar.activation(e[:], o[:, 65:66], EXP, scale=0.125)
                    lsum = scr.tile([128, 1], F32, tag="lsum", name="lsum")
                    nc.vector.tensor_add(lsum[:], o[:, 64:65], e[:])
                else:
                    lsum = o[:, 64:65]
                rec = scr.tile([128, 1], F32, tag="rec", name="rec")
                nc.vector.reciprocal(rec[:], lsum)
                if h < 2:
                    dst = opk01[qg][:, h * 64:(h + 1) * 64]
                else:
                    dst = opk2[qg][:]
                nc.vector.tensor_scalar_mul(dst, o[:, 0:64], rec[:])
                if corr:
                    f = scr.tile([128, 1], F32, tag="f", name="f")
                    nc.vector.tensor_mul(f[:], e[:], rec[:])
                    nc.vector.scalar_tensor_tensor(
                        dst, vsb[:, qg, h, 0:64], f[:], dst, op0=MUL, op1=ADD)

            tp_pool = [None]
            pj_pool = [None]

            def emit_tp(qg, on_act=False):
                cp = nc.scalar.copy if on_act else nc.vector.tensor_copy
                tp = tp_pool[0].tile([128, 128], BF16, tag="tp", name="tp")
                nc.tensor.transpose(tp[:], opk01[qg][:], ident[:])
                cp(ao01[:, qg * 128:(qg + 1) * 128], tp[:])
                tp2 = tp_pool[0].tile([128, 128], BF16, tag="tp", name="tp2")
                nc.tensor.transpose(tp2[0:64, :], opk2[qg][:], ident[:])
                cp(ao2[:, qg * 128:(qg + 1) * 128], tp2[0:64, :])

            pj_n = [0]

            def emit_pj(half, co, sub, on_act=None, width=512):
                q0 = half * 1024 + sub * width
                qsl = slice(q0, q0 + width)
                pj = pj_pool[0].tile([128, 512], F32, tag="pj", name="pj")[:, 0:width]
                nc.tensor.matmul(pj[:], wp01[:, co * 128:(co + 1) * 128],
                                 ao01[:, qsl], start=True, stop=False)
                nc.tensor.matmul(pj[:], wp2[:, co * 128:(co + 1) * 128],
                                 ao2[:, qsl], start=False, stop=True)
                osb = out_sb.tile([128, 512], BF16, tag="osb", name="osb")[:, 0:width]
                # alternate copy engine for pipelining through the 1-buf pool
                if on_act is None:
                    on_act = pj_n[0] % 2 == 0
                if on_act:
                    nc.scalar.copy(osb[:], pj[:])
                else:
                    nc.vector.tensor_copy(osb[:], pj[:])
                # alternate DMA issue queue (SP / Act both have HWDGE)
                dq = nc.sync if pj_n[0] % 2 == 0 else nc.scalar
                pj_n[0] += 1
                dq.dma_start(yT[co * 128:(co + 1) * 128, qsl], osb[:])

            def stream(h, half, fillers):
                """Emit the 12 S^T tiles of one (head, half) interleaved with
                filler thunks (one per tile, remainder after)."""
                fillers = list(fillers)
                for t in range(T):
                    emit_st(h, half, t)
                    if t < len(fillers):
                        fillers[t]()
                for f in fillers[T:]:
                    f()

            AV = lambda h, half, qt: (lambda: emit_av(h, half, qt))
            QK = lambda fi, ch: (lambda: emit_qk(fi, ch))
            V = lambda nt: (lambda: emit_v(nt))
            TP = lambda qg: (lambda: emit_tp(qg))
            PJ = lambda half, co, sub: (lambda: emit_pj(half, co, sub))

            # ---- emission schedule ----
            # Units in order: A=(0,0) B=(1,0) C=(2,0) D=(2,1) E=(1,1) F=(0,1).
            with tc.tile_pool(name="st_ps", bufs=2, space="PSUM") as stps:
                st_pool[0] = stps
                with tc.tile_pool(name="qkv_ps", bufs=2, space="PSUM") as qkv_ps:
                    qkv_pool[0] = qkv_ps
                    emit_qk(1, 0)   # k01 ch0
                    emit_qk(0, 0)   # q01 ch0
                    emit_qk(0, 1)   # q01 ch1 -> S^T(0,0) t0-3 can start
                    stream(0, 0, [V(0), QK(1, 1), V(1), V(2),
                                  QK(1, 2), QK(0, 2), QK(2, 0), QK(2, 1),
                                  V(3), V(4), V(5), V(6)])
                    stream(1, 0, [V(7), V(8), V(9), V(10), V(11)]
                                 + [AV(0, 0, q) for q in range(8)])
                    stream(2, 0, [QK(2, 2)] + [AV(1, 0, q) for q in range(8)]
                                 + [QK(0, 3), QK(1, 3), QK(2, 3)])
                    stream(2, 1, [AV(2, 0, q) for q in range(8)]
                                 + [V(12), V(13), V(14), V(15)])

                with tc.tile_pool(name="tp_ps", bufs=1, space="PSUM") as tp_ps, \
                     tc.tile_pool(name="pj_ps", bufs=1, space="PSUM") as pj_ps:
                    tp_pool[0] = tp_ps
                    pj_pool[0] = pj_ps
                    stream(1, 1, [TP(q) for q in range(8)]
                                 + [PJ(0, 0, 0), PJ(0, 1, 0), PJ(0, 2, 0),
                                    PJ(0, 3, 0)])
                    stream(0, 1, [AV(2, 1, q) for q in range(8)]
                                 + [PJ(0, 4, 0), PJ(0, 5, 0), PJ(0, 0, 1),
                                    PJ(0, 1, 1)])
                    # Act drains exps of F=(0,1); PE: A@V(E), rest of proj h0
                    for i, q in enumerate(range(8)):
                        emit_av(1, 1, q)
                        if i < 4:
                            emit_pj(0, (2, 3, 4, 5)[i], 1)

            # tail: S^T pool closed -> wider transpose/proj pools; exp stream
            # is done so ScalarE takes the proj copies.
            with tc.tile_pool(name="tp2_ps", bufs=2, space="PSUM") as tp2_ps, \
                 tc.tile_pool(name="pj2_ps", bufs=4, space="PSUM") as pj2_ps:
                tp_pool[0] = tp2_ps
                pj_pool[0] = pj2_ps
                for q in range(4):
                    emit_av(0, 1, q)
                    emit_tp(8 + q)
                for q in range(4, 8):
                    emit_av(0, 1, q)
                    emit_tp(8 + q)
                    emit_pj(1, q - 4, 0, on_act=True)
                emit_pj(1, 4, 0, on_act=True)
                emit_pj(1, 5, 0, on_act=False)
                for co in range(6):
                    emit_pj(1, co, 1, on_act=(co % 2 == 0))

    nc.compile()
    return nc


def kernel(**inputs):
    global _last_results
    import ml_dtypes
    from concourse.bass_utils import run_bass_kernel_spmd

    bf16 = ml_dtypes.bfloat16
    x = np.asarray(inputs["x"], np.float32)
    w_qkv = np.asarray(inputs["w_qkv"], np.float32)
    w_proj = np.asarray(inputs["w_proj"], np.float32)
    b_proj = np.asarray(inputs["b_proj"], np.float32)
    u = int(np.asarray(inputs["unseen_size"]))
    B = x.shape[0]

    nc = _build(u)

    wT = np.ascontiguousarray(w_qkv.T)         # [768, 2304]
    wpT_full = np.ascontiguousarray(w_proj.T)  # [768, 768] (ci, co)
    xTb = [np.ascontiguousarray(x[b].T).astype(bf16) for b in range(B)]

    in_maps = []
    for core in range(8):
        b, g = divmod(core, 4)
        hs = [3 * g, 3 * g + 1, 3 * g + 2]
        qcols = [0 * C + h * D + i for h in hs[:2] for i in range(D)]
        kcols = [1 * C + h * D + i for h in hs[:2] for i in range(D)]
        q2 = [0 * C + hs[2] * D + i for i in range(D)]
        k2 = [1 * C + hs[2] * D + i for i in range(D)]
        vcols = [2 * C + h * D + i for h in hs for i in range(D)]
        wqkTc = np.ascontiguousarray(wT[:, qcols + kcols + q2 + k2]).astype(bf16)
        wvTc = np.ascontiguousarray(wT[:, vcols]).astype(bf16)
        ci = [h * D + i for h in hs for i in range(D)]
        wpTc = np.ascontiguousarray(wpT_full[ci, :]).astype(bf16)
        in_maps.append({"xT": xTb[b], "wqkT": wqkTc, "wvT": wvTc, "wpT": wpTc})

    trace = bool(int(os.environ.get("KERNEL_TRACE", "0")))
    res = run_bass_kernel_spmd(nc, in_maps, core_ids=list(range(8)), trace=trace)
    _last_results = res

    y = np.zeros((B, N, C), np.float32)
    for core in range(8):
        b = core // 4
        y[b] += res.results[core]["yT"].T.astype(np.float32)
    y += b_proj
    return y
